# revision 15
# baseline (speedup 1.0000x reference)
"""Trainium2 Bass kernel for nn_ContextualNodeModel (GNN message passing).

Strategy (v3): edge-parallel sharding by destination-node ownership with
host-side pre-gathering and one-hot routing streams. Nodes are dealt into 50
chunk-groups of 1000; a greedy 4-list degree balancer assigns nodes to the 8
cores within each group so per-(list, chunk) contribution counts are nearly
equal across cores and the shared SPMD program wastes little padding.

Per (chunk, list) segment, the host packs:
  - xr stream [128, 2, ns]: remote endpoint features, PE-ready k-tiles (bf16)
  - G stream  [128, ns] fp8: one-hot G[j, e] = (dst-slot(e) == j), so the
    LOCAL endpoint contribution is a single matmul against Pl = x_chunk @
    Wloc (projected on device, 2 matmuls/chunk into the hT PSUM ring) rather
    than two streamed k-tiles -- cuts layer-1 from 10 to 8 columns/slot.
  - S stream  [128 e, nt*128] fp8: per-128-tile scatter one-hot (S = G^T with
    zero rows for pads), consumed directly by the segment-sum matmul so no
    vector-engine is_equal sits in front of the PE (TRN2's PE drops to half
    clock for ~3us after any idle gap, so PE-blocking deps are expensive).

Edge MLP layer-1 accumulates W^T X^T in PSUM (remote k-tiles + G-gathered
local projection + attr term), relu+bias on ACT, layer-2 into [e, f] tiles,
scatter-sum via S into per-chunk aggregates, then the node-local total-flow
MLP. No collectives; output rows are scattered back on the host.
"""
import os
import sys

sys.path.insert(0, "/opt/trn_rl_repo")

import numpy as np
import ml_dtypes

N_NODES = 50000
N_EDGES = 200000
N_FRAME_EDGES = 100000
N_CORES = 8
CHUNK = 128
N_CHUNKS = 50
SLOTS_PER_CORE = N_CHUNKS * CHUNK            # 6400
D = 256
D_EDGE = 32
PAD_REL = -1000.0
LISTS = ("fwd", "bwd", "frE", "frL")
SEC = {"fwd": 0, "frE": 1, "frL": 1, "bwd": 2}
ROUND_SLOTS = 512
FP8_ONE = 0x38                                # float8_e4m3 bit pattern of 1.0

LAST_RESULTS = {}                             # stash for test harness


# ----------------------------------------------------------------- planning
def _assign_nodes(deg):
    """deg [4, N]: per-list destination degree. Snake-deal nodes (by total
    degree) into 50 chunk-groups of 1000, then greedily split each group
    across the 8 cores balancing all 4 per-list degree sums."""
    tot = deg.sum(axis=0).astype(np.int64)
    order = np.argsort(-tot, kind="stable")
    idx = np.arange(N_NODES)
    rounds_i = idx // N_CHUNKS
    pos = idx % N_CHUNKS
    grp = np.where(rounds_i % 2 == 0, pos, N_CHUNKS - 1 - pos)
    group = np.empty(N_NODES, np.int32)
    group[order] = grp.astype(np.int32)

    node_perm = np.full((N_CORES, SLOTS_PER_CORE), -1, np.int64)
    degf = deg.astype(np.float64)
    for ch in range(N_CHUNKS):
        nodes = np.nonzero(group == ch)[0]
        nodes = nodes[np.argsort(-tot[nodes], kind="stable")]
        dn = degf[:, nodes].T                       # [g, 4]
        target = np.maximum(dn.sum(axis=0) / N_CORES, 1.0)
        load = np.zeros((N_CORES, 4))
        cnt = np.zeros(N_CORES, np.int64)
        for i in range(len(nodes)):
            cost = ((load + dn[i]) / target).max(axis=1) + 1e-4 * cnt
            cost[cnt >= CHUNK] = np.inf
            c = int(np.argmin(cost))
            node_perm[c, ch * CHUNK + cnt[c]] = nodes[i]
            load[c] += dn[i]
            cnt[c] += 1
    return node_perm


def _build_plan(edge_index, same_frame_edge_index):
    ei = np.asarray(edge_index)
    fi = np.asarray(same_frame_edge_index)
    past, future = ei[0].astype(np.int64), ei[1].astype(np.int64)
    early, later = fi[0].astype(np.int64), fi[1].astype(np.int64)
    # (dst, src, attr_base): attr id = attr_base + edge position
    lists = {"fwd": (future, past, 0), "bwd": (past, future, 0),
             "frE": (early, later, N_EDGES), "frL": (later, early, N_EDGES)}

    deg = np.zeros((4, N_NODES), np.int64)
    for i, L in enumerate(LISTS):
        deg[i] = np.bincount(lists[L][0], minlength=N_NODES)
    node_perm = _assign_nodes(deg)

    node_core = np.empty(N_NODES, np.int32)
    node_slot = np.empty(N_NODES, np.int32)
    for c in range(N_CORES):
        valid = node_perm[c] >= 0
        node_core[node_perm[c][valid]] = c
        node_slot[node_perm[c][valid]] = np.nonzero(valid)[0].astype(np.int32)

    plan = {"node_perm": node_perm, "ns_pad": {}, "tiles": {},
            "chunk_off": {}, "lists": {L: [] for L in LISTS}}
    for L in LISTS:
        dst, src, abase = lists[L]
        dc = node_core[dst]
        dslot = node_slot[dst]
        dchunk = dslot // CHUNK
        counts = np.zeros((N_CORES, N_CHUNKS), np.int64)
        np.add.at(counts, (dc, dchunk), 1)
        ns_pad = np.maximum(16, ((counts.max(axis=0) + 15) // 16) * 16)
        plan["ns_pad"][L] = ns_pad
        plan["tiles"][L] = (ns_pad + CHUNK - 1) // CHUNK
        chunk_off = np.concatenate([[0], np.cumsum(ns_pad)])
        plan["chunk_off"][L] = chunk_off
        n_slots = int(chunk_off[-1])
        for c in range(N_CORES):
            sel = np.nonzero(dc == c)[0]
            ch = dchunk[sel]
            order = np.argsort(ch, kind="stable")
            sel, ch = sel[order], ch[order]
            within = np.zeros(len(sel), np.int64)
            if len(sel):
                brk = np.nonzero(np.diff(ch))[0] + 1
                starts = np.concatenate([[0], brk])
                lens = np.diff(np.concatenate([starts, [len(sel)]]))
                within = np.arange(len(sel)) - np.repeat(starts, lens)
            slotpos = chunk_off[ch] + within
            srcid = np.full(n_slots, N_NODES, np.int64)          # pad -> zero row
            srcid[slotpos] = src[sel]
            attrid = np.full(n_slots, N_EDGES + N_FRAME_EDGES, np.int64)
            attrid[slotpos] = abase + sel
            rel = np.full(n_slots, PAD_REL, np.float32)
            rel[slotpos] = (node_slot[dst[sel]] % CHUNK).astype(np.float32)
            plan["lists"][L].append(
                {"srcid": srcid, "attrid": attrid, "rel": rel})
    return plan


# ----------------------------------------------------------- input packing
def _pack_core_inputs(inputs, plan, c, xpad, attr_all):
    bf16 = ml_dtypes.bfloat16
    fp8 = ml_dtypes.float8_e4m3

    chunk_off = plan["chunk_off"]
    stot = sum(int(chunk_off[L][-1]) for L in LISTS)
    ttot = int(sum(plan["tiles"][L].sum() for L in LISTS))
    jj = np.arange(CHUNK, dtype=np.float32)

    d = {}
    xs = np.zeros((128, 2 * stot), bf16)
    # G and S share one fp8 stream: per chunk [G segs | S segs]
    gs_u8 = np.zeros((128, stot + ttot * CHUNK), np.uint8)
    ats = np.zeros((D_EDGE, stot), bf16)
    col = scol = gcol = 0
    for ch in range(N_CHUNKS):
        goff = gcol
        gcol += sum(int(plan["ns_pad"][L][ch]) for L in LISTS)
        for L in LISTS:
            lp = plan["lists"][L][c]
            o0, o1 = int(chunk_off[L][ch]), int(chunk_off[L][ch + 1])
            ns = o1 - o0
            rel = lp["rel"][o0:o1]
            xr = xpad[lp["srcid"][o0:o1]]            # [ns, 256] f32
            xs[:, col:col + ns] = xr[:, :128].T; col += ns
            xs[:, col:col + ns] = xr[:, 128:].T; col += ns
            gs_u8[:, goff:goff + ns] = np.where(jj[:, None] == rel[None, :],
                                                FP8_ONE, 0).astype(np.uint8)
            goff += ns
            ats[:, scol:scol + ns] = attr_all[lp["attrid"][o0:o1]].T
            scol += ns
            nt = int(plan["tiles"][L][ch])
            rt = np.full((nt * CHUNK,), PAD_REL, np.float32)
            rt[:ns] = rel
            # S[e, (tile, j)] one-hot, zero rows for pads
            sseg = np.where(rt[:, None] == jj[None, :], FP8_ONE, 0).astype(np.uint8)
            gs_u8[:, gcol:gcol + nt * CHUNK] = (
                sseg.reshape(nt, CHUNK, CHUNK).transpose(1, 0, 2).reshape(CHUNK, nt * CHUNK))
            gcol += nt * CHUNK
    d["xstream"] = xs
    d["gsstream"] = gs_u8.view(fp8)
    d["attrs"] = ats

    # transposed own-node features for the on-device local projections
    xlT = np.zeros((128, N_CHUNKS, 2, CHUNK), np.float32)
    for ch in range(N_CHUNKS):
        ids = plan["node_perm"][c][ch * CHUNK:(ch + 1) * CHUNK]
        xc = xpad[np.where(ids >= 0, ids, N_NODES)]  # [128, 256]
        xlT[:, ch, 0, :] = xc[:, :128].T
        xlT[:, ch, 1, :] = xc[:, 128:].T
    d["xlT"] = xlT.reshape(128, N_CHUNKS * 2 * CHUNK).astype(bf16)

    # ---- weights (same for all cores)
    W1 = {"fwd": inputs["Wf1"], "bwd": inputs["Wb1"], "frE": inputs["Wr1"], "frL": inputs["Wr1"]}
    W2 = {"fwd": inputs["Wf2"], "bwd": inputs["Wb2"], "frE": inputs["Wr2"], "frL": inputs["Wr2"]}
    b1 = {"fwd": inputs["bf1"], "bwd": inputs["bb1"], "frE": inputs["br1"], "frL": inputs["br1"]}
    b2 = {"fwd": inputs["bf2"], "bwd": inputs["bb2"], "frE": inputs["br2"], "frL": inputs["br2"]}
    Wloc = {"fwd": W1["fwd"][0:D], "bwd": W1["bwd"][0:D],
            "frE": W1["frE"][0:D], "frL": W1["frL"][D:2 * D]}
    Wrem = {"fwd": W1["fwd"][D:2 * D], "bwd": W1["bwd"][D:2 * D],
            "frE": W1["frE"][D:2 * D], "frL": W1["frL"][0:D]}
    Watt = {L: np.asarray(W1[L])[2 * D:] for L in LISTS}

    wrem = np.zeros((128, 4 * 2 * 256), np.float32)
    for i, L in enumerate(LISTS):
        W = np.asarray(Wrem[L], np.float32)
        for kb in range(2):
            wrem[:, (i * 2 + kb) * 256:(i * 2 + kb + 1) * 256] = W[kb * 128:(kb + 1) * 128]
    d["Wrem"] = wrem.astype(bf16)
    wloc = np.zeros((128, 2, 4 * 256), np.float32)   # [feat, kb, (L, h)]
    for i, L in enumerate(LISTS):
        W = np.asarray(Wloc[L], np.float32)
        for kb in range(2):
            wloc[:, kb, i * 256:(i + 1) * 256] = W[kb * 128:(kb + 1) * 128]
    d["WlocAll"] = wloc.reshape(128, 2 * 4 * 256).astype(bf16)
    wa = np.zeros((D_EDGE, 4 * 256), np.float32)
    for i, L in enumerate(LISTS):
        wa[:, i * 256:(i + 1) * 256] = np.asarray(Watt[L], np.float32)
    d["Watt"] = wa.astype(bf16)
    w2 = np.zeros((128, 4 * 2 * 128), np.float32)
    for i, L in enumerate(LISTS):
        W = np.asarray(W2[L], np.float32)            # [256, 128]
        for hb in range(2):
            w2[:, (i * 2 + hb) * 128:(i * 2 + hb + 1) * 128] = W[hb * 128:(hb + 1) * 128]
    d["W2"] = w2.astype(bf16)
    b1p = np.zeros((128, 8), np.float32)
    for i, L in enumerate(LISTS):
        bb = np.asarray(b1[L], np.float32)
        for hb in range(2):
            b1p[:, i * 2 + hb] = bb[hb * 128:(hb + 1) * 128]
    d["b1"] = b1p
    b2p = np.zeros((128, 4 * 512), np.float32)
    for i, L in enumerate(LISTS):
        b2p[:, i * 512:(i + 1) * 512] = np.tile(np.asarray(b2[L], np.float32), 4)[None, :]
    d["b2bc"] = b2p
    wt1 = np.zeros((128, 3 * 512), np.float32)
    Wt1 = np.asarray(inputs["Wt1"], np.float32)      # [384, 512]
    for kb in range(3):
        wt1[:, kb * 512:(kb + 1) * 512] = Wt1[kb * 128:(kb + 1) * 128]
    d["Wt1"] = wt1.astype(bf16)
    wt2 = np.zeros((128, 4 * 256), np.float32)
    Wt2 = np.asarray(inputs["Wt2"], np.float32)      # [512, 256]
    for hb in range(4):
        wt2[:, hb * 256:(hb + 1) * 256] = Wt2[hb * 128:(hb + 1) * 128]
    d["Wt2"] = wt2.astype(bf16)
    bt1p = np.zeros((128, 4), np.float32)
    bt1 = np.asarray(inputs["bt1"], np.float32)
    for hb in range(4):
        bt1p[:, hb] = bt1[hb * 128:(hb + 1) * 128]
    d["bt1"] = bt1p
    d["bt2bc"] = np.tile(np.asarray(inputs["bt2"], np.float32)[None, :], (128, 1)).astype(np.float32)
    return d


# ------------------------------------------------------------ bass program
def _build_bass(plan, shapes):
    import concourse.bacc as bacc
    import concourse.tile as tile
    import concourse.mybir as mybir

    bf = mybir.dt.bfloat16
    f32 = mybir.dt.float32
    f8 = mybir.dt.float8e4

    nc = bacc.Bacc("TRN2", target_bir_lowering=False)
    dr = {}
    for name, (shape, dt) in shapes.items():
        kind = "ExternalOutput" if name == "out" else "ExternalInput"
        dr[name] = nc.dram_tensor(name, list(shape), dt, kind=kind)

    ns_pad = plan["ns_pad"]
    tiles = plan["tiles"]

    # per-chunk offsets into streams
    soff = [0]
    toff = [0]
    for ch in range(N_CHUNKS):
        soff.append(soff[-1] + sum(int(ns_pad[L][ch]) for L in LISTS))
        toff.append(toff[-1] + sum(int(tiles[L][ch]) for L in LISTS))

    with tile.TileContext(nc) as tc:
        with (
            tc.tile_pool(name="const", bufs=1) as cpool,
            tc.tile_pool(name="gx", bufs=3) as gxpool,
            tc.tile_pool(name="work", bufs=2) as wpool,
            tc.tile_pool(name="ps_hT", bufs=2, space="PSUM") as ps_hT,
            tc.tile_pool(name="ps_F", bufs=1, space="PSUM") as ps_F,
            tc.tile_pool(name="ps_agg", bufs=1, space="PSUM") as ps_agg,
            tc.tile_pool(name="ps_m2", bufs=1, space="PSUM") as ps_m2,
        ):
            # resident constants
            def cload(name, dt):
                t = cpool.tile(list(shapes[name][0]), dt, tag=name)
                nc.sync.dma_start(t[:], dr[name][:])
                return t

            xlT_sb = cload("xlT", bf)
            Wrem_sb = cload("Wrem", bf)
            Wloc_sb = cload("WlocAll", bf)
            Watt_sb = cload("Watt", bf)
            W2_sb = cload("W2", bf)
            b1_sb = cload("b1", f32)
            b2bc_sb = cload("b2bc", f32)
            Wt1_sb = cload("Wt1", bf)
            Wt2_sb = cload("Wt2", bf)
            bt1_sb = cload("bt1", f32)
            bt2bc_sb = cload("bt2bc", f32)


            # zero-fill hTs ring so partial-tile reads past rn stay finite
            for _ in range(2):
                t0 = wpool.tile([128, 2, ROUND_SLOTS], bf, tag="hTs")
                nc.vector.memset(t0[:], 0.0)

            li = {L: i for i, L in enumerate(LISTS)}

            def emit_pl(ch):
                # local projections Pl[j, (L, h)] = x_chunk @ Wloc, in the hT ring
                Pl_sb = wpool.tile([128, 4 * 256], bf, tag="Pl")
                for half in range(2):
                    Plp = ps_hT.tile([128, 512], f32, tag="hT")
                    for kb in range(2):
                        nc.tensor.matmul(
                            Plp[:],
                            xlT_sb[:, (ch * 2 + kb) * CHUNK:(ch * 2 + kb + 1) * CHUNK],
                            Wloc_sb[:, kb * 1024 + half * 512:kb * 1024 + (half + 1) * 512],
                            start=(kb == 0), stop=(kb == 1))
                    nc.vector.tensor_copy(out=Pl_sb[:, half * 512:(half + 1) * 512],
                                          in_=Plp[:])
                return Pl_sb

            next_Pl = emit_pl(0)

            for ch in range(N_CHUNKS):
                scols = soff[ch + 1] - soff[ch]
                tcols = (toff[ch + 1] - toff[ch]) * CHUNK
                gsbase = soff[ch] + toff[ch] * CHUNK
                X = gxpool.tile([128, 2 * scols], bf, tag="X")
                xb = 0
                for L in LISTS:                      # per-list pieces: finer deps
                    nsL = 2 * int(ns_pad[L][ch])
                    nc.sync.dma_start(X[:, xb:xb + nsL],
                                      dr["xstream"][:, 2 * soff[ch] + xb:2 * soff[ch] + xb + nsL])
                    xb += nsL
                GS = gxpool.tile([128, scols + tcols], f8, tag="GS")
                nc.sync.dma_start(GS[:, :scols], dr["gsstream"][:, gsbase:gsbase + scols])
                nc.sync.dma_start(GS[:, scols:], dr["gsstream"][:, gsbase + scols:gsbase + scols + tcols])
                A = gxpool.tile([D_EDGE, scols], bf, tag="A")
                nc.sync.dma_start(A[:], dr["attrs"][:, soff[ch]:soff[ch + 1]])
                Pl_sb = next_Pl

                aggT = ps_agg.tile([128, 3, 128], f32, tag="aggT")
                sec_first = {0: True, 1: True, 2: True}
                n_sec_tiles = {0: int(tiles["fwd"][ch]),
                               1: int(tiles["frE"][ch] + tiles["frL"][ch]),
                               2: int(tiles["bwd"][ch])}
                sec_done = {0: 0, 1: 0, 2: 0}

                xbase = 0
                sbase = 0
                tbase = 0
                for Li, L in enumerate(LISTS):
                    iL = li[L]
                    ns = int(ns_pad[L][ch])
                    nt = int(tiles[L][ch])
                    sec = SEC[L]

                    for r0 in range(0, ns, ROUND_SLOTS):
                        rn = min(ROUND_SLOTS, ns - r0)           # slots in round
                        rt = (rn + CHUNK - 1) // CHUNK           # tiles in round
                        rc = rt * CHUNK                          # f-cols in round
                        hT = ps_hT.tile([128, 2, ROUND_SLOTS], f32, tag="hT")
                        for hb in range(2):
                            for kb in range(2):
                                nc.tensor.matmul(
                                    hT[:, hb, :rn],
                                    Wrem_sb[:, (iL * 2 + kb) * 256 + hb * 128:(iL * 2 + kb) * 256 + hb * 128 + 128],
                                    X[:, xbase + kb * ns + r0:xbase + kb * ns + r0 + rn],
                                    start=(kb == 0), stop=False)
                            nc.tensor.matmul(
                                hT[:, hb, :rn],
                                Pl_sb[:, iL * 256 + hb * 128:iL * 256 + hb * 128 + 128],
                                GS[:, sbase + r0:sbase + r0 + rn],
                                start=False, stop=False)
                            nc.tensor.matmul(
                                hT[:, hb, :rn],
                                Watt_sb[:, iL * 256 + hb * 128:iL * 256 + hb * 128 + 128],
                                A[:, sbase + r0:sbase + r0 + rn],
                                start=False, stop=True)
                        hTs = wpool.tile([128, 2, ROUND_SLOTS], bf, tag="hTs")
                        for hb in range(2):
                            nc.scalar.activation(
                                hTs[:, hb, :rn], hT[:, hb, :rn],
                                mybir.ActivationFunctionType.Relu,
                                bias=b1_sb[:, iL * 2 + hb:iL * 2 + hb + 1])
                        Fp = ps_F.tile([128, ROUND_SLOTS], f32, tag="F")
                        for i in range(rt):
                            for hb in range(2):
                                nc.tensor.matmul(
                                    Fp[:, i * 128:(i + 1) * 128],
                                    hTs[:, hb, i * 128:(i + 1) * 128],
                                    W2_sb[:, (iL * 2 + hb) * 128:(iL * 2 + hb + 1) * 128],
                                    start=(hb == 0), stop=(hb == 1))
                        Fs = wpool.tile([128, ROUND_SLOTS], bf, tag="Fs")
                        nc.vector.tensor_tensor(
                            out=Fs[:, :rc], in0=Fp[:, :rc],
                            in1=b2bc_sb[:, iL * 512:iL * 512 + rc],
                            op=mybir.AluOpType.add)
                        for i in range(rt):
                            ti = tbase + r0 // CHUNK + i
                            first = sec_first[sec]
                            sec_first[sec] = False
                            sec_done[sec] += 1
                            nc.tensor.matmul(
                                aggT[:, sec, :],
                                Fs[:, i * 128:(i + 1) * 128],
                                GS[:, scols + ti * CHUNK:scols + (ti + 1) * CHUNK],
                                start=first, stop=(sec_done[sec] == n_sec_tiles[sec]))
                    if Li == 0 and ch + 1 < N_CHUNKS:
                        next_Pl = emit_pl(ch + 1)
                    xbase += 2 * ns
                    sbase += ns
                    tbase += nt

                # ---- total-flow MLP, batched over chunk pairs
                if ch % 2 == 0:
                    aggTs = wpool.tile([128, 3, 2, 128], bf, tag="aggTs")
                    pair_aggTs = aggTs
                for kb in range(3):
                    nc.vector.tensor_copy(out=pair_aggTs[:, kb, ch % 2, :],
                                          in_=aggT[:, kb, :])
                if ch % 2 == 1:
                    h2 = ps_m2.tile([128, 4, 256], f32, tag="m2")
                    for hb in range(4):
                        for kb in range(3):
                            nc.tensor.matmul(
                                h2[:, hb, :],
                                Wt1_sb[:, kb * 512 + hb * 128:kb * 512 + hb * 128 + 128],
                                pair_aggTs[:, kb, :, :],
                                start=(kb == 0), stop=(kb == 2))
                    h2s = wpool.tile([128, 4, 256], bf, tag="h2s")
                    for hb in range(4):
                        nc.scalar.activation(
                            h2s[:, hb, :], h2[:, hb, :],
                            mybir.ActivationFunctionType.Relu,
                            bias=bt1_sb[:, hb:hb + 1])
                    op2 = ps_m2.tile([128, 2, 256], f32, tag="m2")
                    for cc in range(2):
                        for hb in range(4):
                            nc.tensor.matmul(
                                op2[:, cc, :], h2s[:, hb, cc * 128:(cc + 1) * 128],
                                Wt2_sb[:, hb * 256:(hb + 1) * 256],
                                start=(hb == 0), stop=(hb == 3))
                        outs = wpool.tile([128, 256], f32, tag="outs")
                        nc.vector.tensor_tensor(out=outs[:], in0=op2[:, cc, :], in1=bt2bc_sb[:],
                                                op=mybir.AluOpType.add)
                        nc.sync.dma_start(dr["out"][ch - 1 + cc], outs[:])

    nc.compile()
    return nc


# ----------------------------------------------------------------- kernel
def kernel(**inputs):
    import concourse.mybir as mybir
    from concourse.bass_utils import run_bass_kernel_spmd

    bf = mybir.dt.bfloat16
    f32 = mybir.dt.float32
    f8 = mybir.dt.float8e4

    plan = _build_plan(np.asarray(inputs["edge_index"]),
                       np.asarray(inputs["same_frame_edge_index"]))

    x = np.asarray(inputs["x"], np.float32)
    xpad = np.vstack([x, np.zeros((1, D), np.float32)])
    ea = np.asarray(inputs["edge_attr"], np.float32)
    fa = np.asarray(inputs["same_frame_edge_attr"], np.float32)
    attr_all = np.vstack([ea, fa, np.zeros((1, D_EDGE), np.float32)])

    cores = [_pack_core_inputs(inputs, plan, c, xpad, attr_all)
             for c in range(N_CORES)]

    shapes = {}
    for name, arr in cores[0].items():
        dt = {np.dtype(np.float32): f32,
              np.dtype(ml_dtypes.bfloat16): bf,
              np.dtype(ml_dtypes.float8_e4m3): f8}[arr.dtype]
        shapes[name] = (arr.shape, dt)
    shapes["out"] = ((N_CHUNKS, 128, 256), f32)

    nc = _build_bass(plan, shapes)

    trace = bool(int(os.environ.get("GNN_TRACE", "0")))
    res = run_bass_kernel_spmd(nc, cores, core_ids=list(range(N_CORES)),
                               trace=trace)
    LAST_RESULTS["res"] = res

    out = np.zeros((N_NODES, 256), np.float32)
    for c in range(N_CORES):
        oc = np.asarray(res.results[c]["out"], np.float32).reshape(SLOTS_PER_CORE, 256)
        valid = plan["node_perm"][c] >= 0
        out[plan["node_perm"][c][valid]] = oc[valid]
    return out


# revision 16
# speedup vs baseline: 1.0064x; 1.0064x over previous
"""Trainium2 Bass kernel for nn_ContextualNodeModel (GNN message passing).

Strategy (v3): edge-parallel sharding by destination-node ownership with
host-side pre-gathering and one-hot routing streams. Nodes are dealt into 50
chunk-groups of 1000; a greedy 4-list degree balancer assigns nodes to the 8
cores within each group so per-(list, chunk) contribution counts are nearly
equal across cores and the shared SPMD program wastes little padding.

Per (chunk, list) segment, the host packs:
  - xr stream [128, 2, ns]: remote endpoint features, PE-ready k-tiles (bf16)
  - G stream  [128, ns] fp8: one-hot G[j, e] = (dst-slot(e) == j), so the
    LOCAL endpoint contribution is a single matmul against Pl = x_chunk @
    Wloc (projected on device, 2 matmuls/chunk into the hT PSUM ring) rather
    than two streamed k-tiles -- cuts layer-1 from 10 to 8 columns/slot.
  - S stream  [128 e, nt*128] fp8: per-128-tile scatter one-hot (S = G^T with
    zero rows for pads), consumed directly by the segment-sum matmul so no
    vector-engine is_equal sits in front of the PE (TRN2's PE drops to half
    clock for ~3us after any idle gap, so PE-blocking deps are expensive).

Edge MLP layer-1 accumulates W^T X^T in PSUM (remote k-tiles + G-gathered
local projection + attr term), relu+bias on ACT, layer-2 into [e, f] tiles,
scatter-sum via S into per-chunk aggregates, then the node-local total-flow
MLP. No collectives; output rows are scattered back on the host.
"""
import os
import sys

sys.path.insert(0, "/opt/trn_rl_repo")

import numpy as np
import ml_dtypes

N_NODES = 50000
N_EDGES = 200000
N_FRAME_EDGES = 100000
N_CORES = 8
CHUNK = 128
N_CHUNKS = 50
SLOTS_PER_CORE = N_CHUNKS * CHUNK            # 6400
D = 256
D_EDGE = 32
PAD_REL = -1000.0
LISTS = ("fwd", "bwd", "frE", "frL")
SEC = {"fwd": 0, "frE": 1, "frL": 1, "bwd": 2}
ROUND_SLOTS = 512
FP8_ONE = 0x38                                # float8_e4m3 bit pattern of 1.0

LAST_RESULTS = {}                             # stash for test harness


# ----------------------------------------------------------------- planning
def _assign_nodes(deg):
    """deg [4, N]: per-list destination degree. Snake-deal nodes (by total
    degree) into 50 chunk-groups of 1000, then greedily split each group
    across the 8 cores balancing all 4 per-list degree sums."""
    tot = deg.sum(axis=0).astype(np.int64)
    order = np.argsort(-tot, kind="stable")
    idx = np.arange(N_NODES)
    rounds_i = idx // N_CHUNKS
    pos = idx % N_CHUNKS
    grp = np.where(rounds_i % 2 == 0, pos, N_CHUNKS - 1 - pos)
    group = np.empty(N_NODES, np.int32)
    group[order] = grp.astype(np.int32)

    node_perm = np.full((N_CORES, SLOTS_PER_CORE), -1, np.int64)
    degf = deg.astype(np.float64)
    for ch in range(N_CHUNKS):
        nodes = np.nonzero(group == ch)[0]
        nodes = nodes[np.argsort(-tot[nodes], kind="stable")]
        dn = degf[:, nodes].T                       # [g, 4]
        target = np.maximum(dn.sum(axis=0) / N_CORES, 1.0)
        load = np.zeros((N_CORES, 4))
        cnt = np.zeros(N_CORES, np.int64)
        for i in range(len(nodes)):
            cost = ((load + dn[i]) / target).max(axis=1) + 1e-4 * cnt
            cost[cnt >= CHUNK] = np.inf
            c = int(np.argmin(cost))
            node_perm[c, ch * CHUNK + cnt[c]] = nodes[i]
            load[c] += dn[i]
            cnt[c] += 1
    return node_perm


def _build_plan(edge_index, same_frame_edge_index):
    ei = np.asarray(edge_index)
    fi = np.asarray(same_frame_edge_index)
    past, future = ei[0].astype(np.int64), ei[1].astype(np.int64)
    early, later = fi[0].astype(np.int64), fi[1].astype(np.int64)
    # (dst, src, attr_base): attr id = attr_base + edge position
    lists = {"fwd": (future, past, 0), "bwd": (past, future, 0),
             "frE": (early, later, N_EDGES), "frL": (later, early, N_EDGES)}

    deg = np.zeros((4, N_NODES), np.int64)
    for i, L in enumerate(LISTS):
        deg[i] = np.bincount(lists[L][0], minlength=N_NODES)
    node_perm = _assign_nodes(deg)

    node_core = np.empty(N_NODES, np.int32)
    node_slot = np.empty(N_NODES, np.int32)
    for c in range(N_CORES):
        valid = node_perm[c] >= 0
        node_core[node_perm[c][valid]] = c
        node_slot[node_perm[c][valid]] = np.nonzero(valid)[0].astype(np.int32)

    plan = {"node_perm": node_perm, "ns_pad": {}, "tiles": {},
            "chunk_off": {}, "lists": {L: [] for L in LISTS}}
    for L in LISTS:
        dst, src, abase = lists[L]
        dc = node_core[dst]
        dslot = node_slot[dst]
        dchunk = dslot // CHUNK
        counts = np.zeros((N_CORES, N_CHUNKS), np.int64)
        np.add.at(counts, (dc, dchunk), 1)
        ns_pad = np.maximum(16, ((counts.max(axis=0) + 15) // 16) * 16)
        plan["ns_pad"][L] = ns_pad
        plan["tiles"][L] = (ns_pad + CHUNK - 1) // CHUNK
        chunk_off = np.concatenate([[0], np.cumsum(ns_pad)])
        plan["chunk_off"][L] = chunk_off
        n_slots = int(chunk_off[-1])
        for c in range(N_CORES):
            sel = np.nonzero(dc == c)[0]
            ch = dchunk[sel]
            order = np.argsort(ch, kind="stable")
            sel, ch = sel[order], ch[order]
            within = np.zeros(len(sel), np.int64)
            if len(sel):
                brk = np.nonzero(np.diff(ch))[0] + 1
                starts = np.concatenate([[0], brk])
                lens = np.diff(np.concatenate([starts, [len(sel)]]))
                within = np.arange(len(sel)) - np.repeat(starts, lens)
            slotpos = chunk_off[ch] + within
            srcid = np.full(n_slots, N_NODES, np.int64)          # pad -> zero row
            srcid[slotpos] = src[sel]
            attrid = np.full(n_slots, N_EDGES + N_FRAME_EDGES, np.int64)
            attrid[slotpos] = abase + sel
            rel = np.full(n_slots, PAD_REL, np.float32)
            rel[slotpos] = (node_slot[dst[sel]] % CHUNK).astype(np.float32)
            plan["lists"][L].append(
                {"srcid": srcid, "attrid": attrid, "rel": rel})
    return plan


# ----------------------------------------------------------- input packing
def _pack_core_inputs(inputs, plan, c, xpad, attr_all):
    bf16 = ml_dtypes.bfloat16
    fp8 = ml_dtypes.float8_e4m3

    chunk_off = plan["chunk_off"]
    stot = sum(int(chunk_off[L][-1]) for L in LISTS)
    ttot = int(sum(plan["tiles"][L].sum() for L in LISTS))
    jj = np.arange(CHUNK, dtype=np.float32)

    d = {}
    xs = np.zeros((128, 2 * stot), bf16)
    # G and S share one fp8 stream: per chunk [G segs | S segs]
    gs_u8 = np.zeros((128, stot + ttot * CHUNK), np.uint8)
    ats = np.zeros((D_EDGE, stot), bf16)
    col = scol = gcol = 0
    for ch in range(N_CHUNKS):
        goff = gcol
        gcol += sum(int(plan["ns_pad"][L][ch]) for L in LISTS)
        for L in LISTS:
            lp = plan["lists"][L][c]
            o0, o1 = int(chunk_off[L][ch]), int(chunk_off[L][ch + 1])
            ns = o1 - o0
            rel = lp["rel"][o0:o1]
            xr = xpad[lp["srcid"][o0:o1]]            # [ns, 256] f32
            xs[:, col:col + ns] = xr[:, :128].T; col += ns
            xs[:, col:col + ns] = xr[:, 128:].T; col += ns
            gs_u8[:, goff:goff + ns] = np.where(jj[:, None] == rel[None, :],
                                                FP8_ONE, 0).astype(np.uint8)
            goff += ns
            ats[:, scol:scol + ns] = attr_all[lp["attrid"][o0:o1]].T
            scol += ns
            nt = int(plan["tiles"][L][ch])
            rt = np.full((nt * CHUNK,), PAD_REL, np.float32)
            rt[:ns] = rel
            # S[e, (tile, j)] one-hot, zero rows for pads
            sseg = np.where(rt[:, None] == jj[None, :], FP8_ONE, 0).astype(np.uint8)
            gs_u8[:, gcol:gcol + nt * CHUNK] = (
                sseg.reshape(nt, CHUNK, CHUNK).transpose(1, 0, 2).reshape(CHUNK, nt * CHUNK))
            gcol += nt * CHUNK
    d["xstream"] = xs
    d["gsstream"] = gs_u8.view(fp8)
    d["attrs"] = ats

    # transposed own-node features for the on-device local projections
    xlT = np.zeros((128, N_CHUNKS, 2, CHUNK), np.float32)
    for ch in range(N_CHUNKS):
        ids = plan["node_perm"][c][ch * CHUNK:(ch + 1) * CHUNK]
        xc = xpad[np.where(ids >= 0, ids, N_NODES)]  # [128, 256]
        xlT[:, ch, 0, :] = xc[:, :128].T
        xlT[:, ch, 1, :] = xc[:, 128:].T
    d["xlT"] = xlT.reshape(128, N_CHUNKS * 2 * CHUNK).astype(bf16)

    # ---- weights (same for all cores)
    W1 = {"fwd": inputs["Wf1"], "bwd": inputs["Wb1"], "frE": inputs["Wr1"], "frL": inputs["Wr1"]}
    W2 = {"fwd": inputs["Wf2"], "bwd": inputs["Wb2"], "frE": inputs["Wr2"], "frL": inputs["Wr2"]}
    b1 = {"fwd": inputs["bf1"], "bwd": inputs["bb1"], "frE": inputs["br1"], "frL": inputs["br1"]}
    b2 = {"fwd": inputs["bf2"], "bwd": inputs["bb2"], "frE": inputs["br2"], "frL": inputs["br2"]}
    Wloc = {"fwd": W1["fwd"][0:D], "bwd": W1["bwd"][0:D],
            "frE": W1["frE"][0:D], "frL": W1["frL"][D:2 * D]}
    Wrem = {"fwd": W1["fwd"][D:2 * D], "bwd": W1["bwd"][D:2 * D],
            "frE": W1["frE"][D:2 * D], "frL": W1["frL"][0:D]}
    Watt = {L: np.asarray(W1[L])[2 * D:] for L in LISTS}

    wrem = np.zeros((128, 4 * 2 * 256), np.float32)
    for i, L in enumerate(LISTS):
        W = np.asarray(Wrem[L], np.float32)
        for kb in range(2):
            wrem[:, (i * 2 + kb) * 256:(i * 2 + kb + 1) * 256] = W[kb * 128:(kb + 1) * 128]
    d["Wrem"] = wrem.astype(bf16)
    wloc = np.zeros((128, 2, 4 * 256), np.float32)   # [feat, kb, (L, h)]
    for i, L in enumerate(LISTS):
        W = np.asarray(Wloc[L], np.float32)
        for kb in range(2):
            wloc[:, kb, i * 256:(i + 1) * 256] = W[kb * 128:(kb + 1) * 128]
    d["WlocAll"] = wloc.reshape(128, 2 * 4 * 256).astype(bf16)
    wa = np.zeros((D_EDGE, 4 * 256), np.float32)
    for i, L in enumerate(LISTS):
        wa[:, i * 256:(i + 1) * 256] = np.asarray(Watt[L], np.float32)
    d["Watt"] = wa.astype(bf16)
    w2 = np.zeros((128, 4 * 2 * 128), np.float32)
    for i, L in enumerate(LISTS):
        W = np.asarray(W2[L], np.float32)            # [256, 128]
        for hb in range(2):
            w2[:, (i * 2 + hb) * 128:(i * 2 + hb + 1) * 128] = W[hb * 128:(hb + 1) * 128]
    d["W2"] = w2.astype(bf16)
    b1p = np.zeros((128, 8), np.float32)
    for i, L in enumerate(LISTS):
        bb = np.asarray(b1[L], np.float32)
        for hb in range(2):
            b1p[:, i * 2 + hb] = bb[hb * 128:(hb + 1) * 128]
    d["b1"] = b1p
    b2p = np.zeros((128, 4 * 512), np.float32)
    for i, L in enumerate(LISTS):
        b2p[:, i * 512:(i + 1) * 512] = np.tile(np.asarray(b2[L], np.float32), 4)[None, :]
    d["b2bc"] = b2p
    wt1 = np.zeros((128, 3 * 512), np.float32)
    Wt1 = np.asarray(inputs["Wt1"], np.float32)      # [384, 512]
    for kb in range(3):
        wt1[:, kb * 512:(kb + 1) * 512] = Wt1[kb * 128:(kb + 1) * 128]
    d["Wt1"] = wt1.astype(bf16)
    wt2 = np.zeros((128, 4 * 256), np.float32)
    Wt2 = np.asarray(inputs["Wt2"], np.float32)      # [512, 256]
    for hb in range(4):
        wt2[:, hb * 256:(hb + 1) * 256] = Wt2[hb * 128:(hb + 1) * 128]
    d["Wt2"] = wt2.astype(bf16)
    bt1p = np.zeros((128, 4), np.float32)
    bt1 = np.asarray(inputs["bt1"], np.float32)
    for hb in range(4):
        bt1p[:, hb] = bt1[hb * 128:(hb + 1) * 128]
    d["bt1"] = bt1p
    d["bt2bc"] = np.tile(np.asarray(inputs["bt2"], np.float32)[None, :], (128, 1)).astype(np.float32)
    return d


# ------------------------------------------------------------ bass program
def _build_bass(plan, shapes):
    import concourse.bacc as bacc
    import concourse.tile as tile
    import concourse.mybir as mybir

    bf = mybir.dt.bfloat16
    f32 = mybir.dt.float32
    f8 = mybir.dt.float8e4

    nc = bacc.Bacc("TRN2", target_bir_lowering=False)
    dr = {}
    for name, (shape, dt) in shapes.items():
        kind = "ExternalOutput" if name == "out" else "ExternalInput"
        dr[name] = nc.dram_tensor(name, list(shape), dt, kind=kind)

    ns_pad = plan["ns_pad"]
    tiles = plan["tiles"]

    # per-chunk offsets into streams
    soff = [0]
    toff = [0]
    for ch in range(N_CHUNKS):
        soff.append(soff[-1] + sum(int(ns_pad[L][ch]) for L in LISTS))
        toff.append(toff[-1] + sum(int(tiles[L][ch]) for L in LISTS))

    with tile.TileContext(nc) as tc:
        with (
            tc.tile_pool(name="const", bufs=1) as cpool,
            tc.tile_pool(name="gx", bufs=3) as gxpool,
            tc.tile_pool(name="work", bufs=2) as wpool,
            tc.tile_pool(name="ps_hT", bufs=2, space="PSUM") as ps_hT,
            tc.tile_pool(name="ps_F", bufs=1, space="PSUM") as ps_F,
            tc.tile_pool(name="ps_agg", bufs=1, space="PSUM") as ps_agg,
            tc.tile_pool(name="ps_m2", bufs=1, space="PSUM") as ps_m2,
        ):
            # resident constants
            def cload(name, dt):
                t = cpool.tile(list(shapes[name][0]), dt, tag=name)
                nc.sync.dma_start(t[:], dr[name][:])
                return t

            xlT_sb = cload("xlT", bf)
            Wrem_sb = cload("Wrem", bf)
            Wloc_sb = cload("WlocAll", bf)
            Watt_sb = cload("Watt", bf)
            W2_sb = cload("W2", bf)
            b1_sb = cload("b1", f32)
            b2bc_sb = cload("b2bc", f32)
            Wt1_sb = cload("Wt1", bf)
            Wt2_sb = cload("Wt2", bf)
            bt1_sb = cload("bt1", f32)
            bt2bc_sb = cload("bt2bc", f32)


            # zero-fill hTs ring so partial-tile reads past rn stay finite
            for _ in range(2):
                t0 = wpool.tile([128, 2, ROUND_SLOTS], bf, tag="hTs")
                nc.vector.memset(t0[:], 0.0)

            li = {L: i for i, L in enumerate(LISTS)}

            def emit_pl(ch):
                # local projections Pl[j, (L, h)] = x_chunk @ Wloc, in the hT ring
                Pl_sb = wpool.tile([128, 4 * 256], bf, tag="Pl")
                for half in range(2):
                    Plp = ps_hT.tile([128, 512], f32, tag="hT")
                    for kb in range(2):
                        nc.tensor.matmul(
                            Plp[:],
                            xlT_sb[:, (ch * 2 + kb) * CHUNK:(ch * 2 + kb + 1) * CHUNK],
                            Wloc_sb[:, kb * 1024 + half * 512:kb * 1024 + (half + 1) * 512],
                            start=(kb == 0), stop=(kb == 1))
                    nc.vector.tensor_copy(out=Pl_sb[:, half * 512:(half + 1) * 512],
                                          in_=Plp[:])
                return Pl_sb

            next_Pl = emit_pl(0)

            for ch in range(N_CHUNKS):
                scols = soff[ch + 1] - soff[ch]
                tcols = (toff[ch + 1] - toff[ch]) * CHUNK
                gsbase = soff[ch] + toff[ch] * CHUNK
                X = gxpool.tile([128, 2 * scols], bf, tag="X")
                nc.sync.dma_start(X[:], dr["xstream"][:, 2 * soff[ch]:2 * soff[ch + 1]])
                GS = gxpool.tile([128, scols + tcols], f8, tag="GS")
                nc.sync.dma_start(GS[:], dr["gsstream"][:, gsbase:gsbase + scols + tcols])
                A = gxpool.tile([D_EDGE, scols], bf, tag="A")
                nc.sync.dma_start(A[:], dr["attrs"][:, soff[ch]:soff[ch + 1]])
                Pl_sb = next_Pl

                aggT = ps_agg.tile([128, 3, 128], f32, tag="aggT")
                sec_first = {0: True, 1: True, 2: True}
                n_sec_tiles = {0: int(tiles["fwd"][ch]),
                               1: int(tiles["frE"][ch] + tiles["frL"][ch]),
                               2: int(tiles["bwd"][ch])}
                sec_done = {0: 0, 1: 0, 2: 0}

                xbase = 0
                sbase = 0
                tbase = 0
                for Li, L in enumerate(LISTS):
                    iL = li[L]
                    ns = int(ns_pad[L][ch])
                    nt = int(tiles[L][ch])
                    sec = SEC[L]

                    for r0 in range(0, ns, ROUND_SLOTS):
                        rn = min(ROUND_SLOTS, ns - r0)           # slots in round
                        rt = (rn + CHUNK - 1) // CHUNK           # tiles in round
                        rc = rt * CHUNK                          # f-cols in round
                        hT = ps_hT.tile([128, 2, ROUND_SLOTS], f32, tag="hT")
                        for hb in range(2):
                            for kb in range(2):
                                nc.tensor.matmul(
                                    hT[:, hb, :rn],
                                    Wrem_sb[:, (iL * 2 + kb) * 256 + hb * 128:(iL * 2 + kb) * 256 + hb * 128 + 128],
                                    X[:, xbase + kb * ns + r0:xbase + kb * ns + r0 + rn],
                                    start=(kb == 0), stop=False)
                            nc.tensor.matmul(
                                hT[:, hb, :rn],
                                Pl_sb[:, iL * 256 + hb * 128:iL * 256 + hb * 128 + 128],
                                GS[:, sbase + r0:sbase + r0 + rn],
                                start=False, stop=False)
                            nc.tensor.matmul(
                                hT[:, hb, :rn],
                                Watt_sb[:, iL * 256 + hb * 128:iL * 256 + hb * 128 + 128],
                                A[:, sbase + r0:sbase + r0 + rn],
                                start=False, stop=True)
                        hTs = wpool.tile([128, 2, ROUND_SLOTS], bf, tag="hTs")
                        for hb in range(2):
                            nc.scalar.activation(
                                hTs[:, hb, :rn], hT[:, hb, :rn],
                                mybir.ActivationFunctionType.Relu,
                                bias=b1_sb[:, iL * 2 + hb:iL * 2 + hb + 1])
                        Fp = ps_F.tile([128, ROUND_SLOTS], f32, tag="F")
                        for i in range(rt):
                            for hb in range(2):
                                nc.tensor.matmul(
                                    Fp[:, i * 128:(i + 1) * 128],
                                    hTs[:, hb, i * 128:(i + 1) * 128],
                                    W2_sb[:, (iL * 2 + hb) * 128:(iL * 2 + hb + 1) * 128],
                                    start=(hb == 0), stop=(hb == 1))
                        Fs = wpool.tile([128, ROUND_SLOTS], bf, tag="Fs")
                        nc.vector.tensor_tensor(
                            out=Fs[:, :rc], in0=Fp[:, :rc],
                            in1=b2bc_sb[:, iL * 512:iL * 512 + rc],
                            op=mybir.AluOpType.add)
                        for i in range(rt):
                            ti = tbase + r0 // CHUNK + i
                            first = sec_first[sec]
                            sec_first[sec] = False
                            sec_done[sec] += 1
                            nc.tensor.matmul(
                                aggT[:, sec, :],
                                Fs[:, i * 128:(i + 1) * 128],
                                GS[:, scols + ti * CHUNK:scols + (ti + 1) * CHUNK],
                                start=first, stop=(sec_done[sec] == n_sec_tiles[sec]))
                    if Li == 0 and ch + 1 < N_CHUNKS:
                        next_Pl = emit_pl(ch + 1)
                    xbase += 2 * ns
                    sbase += ns
                    tbase += nt

                # ---- total-flow MLP, batched over chunk pairs
                if ch % 2 == 0:
                    aggTs = wpool.tile([128, 3, 2, 128], bf, tag="aggTs")
                    pair_aggTs = aggTs
                for kb in range(3):
                    nc.vector.tensor_copy(out=pair_aggTs[:, kb, ch % 2, :],
                                          in_=aggT[:, kb, :])
                if ch % 2 == 1:
                    h2 = ps_m2.tile([128, 4, 256], f32, tag="m2")
                    for hb in range(4):
                        for kb in range(3):
                            nc.tensor.matmul(
                                h2[:, hb, :],
                                Wt1_sb[:, kb * 512 + hb * 128:kb * 512 + hb * 128 + 128],
                                pair_aggTs[:, kb, :, :],
                                start=(kb == 0), stop=(kb == 2))
                    h2s = wpool.tile([128, 4, 256], bf, tag="h2s")
                    for hb in range(4):
                        nc.scalar.activation(
                            h2s[:, hb, :], h2[:, hb, :],
                            mybir.ActivationFunctionType.Relu,
                            bias=bt1_sb[:, hb:hb + 1])
                    op2 = ps_m2.tile([128, 2, 256], f32, tag="m2")
                    for cc in range(2):
                        for hb in range(4):
                            nc.tensor.matmul(
                                op2[:, cc, :], h2s[:, hb, cc * 128:(cc + 1) * 128],
                                Wt2_sb[:, hb * 256:(hb + 1) * 256],
                                start=(hb == 0), stop=(hb == 3))
                        outs = wpool.tile([128, 256], f32, tag="outs")
                        nc.vector.tensor_tensor(out=outs[:], in0=op2[:, cc, :], in1=bt2bc_sb[:],
                                                op=mybir.AluOpType.add)
                        nc.sync.dma_start(dr["out"][ch - 1 + cc], outs[:])

    nc.compile()
    return nc


# ----------------------------------------------------------------- kernel
def kernel(**inputs):
    import concourse.mybir as mybir
    from concourse.bass_utils import run_bass_kernel_spmd

    bf = mybir.dt.bfloat16
    f32 = mybir.dt.float32
    f8 = mybir.dt.float8e4

    plan = _build_plan(np.asarray(inputs["edge_index"]),
                       np.asarray(inputs["same_frame_edge_index"]))

    x = np.asarray(inputs["x"], np.float32)
    xpad = np.vstack([x, np.zeros((1, D), np.float32)])
    ea = np.asarray(inputs["edge_attr"], np.float32)
    fa = np.asarray(inputs["same_frame_edge_attr"], np.float32)
    attr_all = np.vstack([ea, fa, np.zeros((1, D_EDGE), np.float32)])

    cores = [_pack_core_inputs(inputs, plan, c, xpad, attr_all)
             for c in range(N_CORES)]

    shapes = {}
    for name, arr in cores[0].items():
        dt = {np.dtype(np.float32): f32,
              np.dtype(ml_dtypes.bfloat16): bf,
              np.dtype(ml_dtypes.float8_e4m3): f8}[arr.dtype]
        shapes[name] = (arr.shape, dt)
    shapes["out"] = ((N_CHUNKS, 128, 256), f32)

    nc = _build_bass(plan, shapes)

    trace = bool(int(os.environ.get("GNN_TRACE", "0")))
    res = run_bass_kernel_spmd(nc, cores, core_ids=list(range(N_CORES)),
                               trace=trace)
    LAST_RESULTS["res"] = res

    out = np.zeros((N_NODES, 256), np.float32)
    for c in range(N_CORES):
        oc = np.asarray(res.results[c]["out"], np.float32).reshape(SLOTS_PER_CORE, 256)
        valid = plan["node_perm"][c] >= 0
        out[plan["node_perm"][c][valid]] = oc[valid]
    return out
